# revision 3
# baseline (speedup 1.0000x reference)
"""Data-parallel Trainium kernel for the 7-DOF arm RK4 forward-dynamics step.

Strategy: pure data parallelism. The batch (32768) is split into 8 shards of
4096, one per NeuronCore; the tiny kinematic parameters are replicated. The
whole physics step (RK4 over forward dynamics + FK) is compiled once per core
via PJRT and executed SPMD with jax.pmap; outputs are concatenated back to the
full batch. No cross-sample communication exists, so this is exact.
"""
import functools

import jax
import jax.numpy as jnp
import numpy as np

DOF = 7
BATCH = 32768
N_CORES = 8
ACTION_RANGE = 50.0
MAX_VEL = 20.0
TIMESTEP = 0.1
DAMPING = 0.0


def _skew(w):
    x, y, z = w[..., 0], w[..., 1], w[..., 2]
    o = jnp.zeros_like(x)
    return jnp.stack([jnp.stack([o, -z, y], -1),
                      jnp.stack([z, o, -x], -1),
                      jnp.stack([-y, x, o], -1)], -2)


def _make_T(R, p):
    Rp = jnp.concatenate([R, p[..., None]], -1)
    bot = jnp.broadcast_to(jnp.array([0., 0., 0., 1.], R.dtype), Rp.shape[:-2] + (1, 4))
    return jnp.concatenate([Rp, bot], -2)


def _inv_SE3(T):
    R = T[..., :3, :3]; p = T[..., :3, 3]
    Rt = jnp.swapaxes(R, -1, -2)
    return _make_T(Rt, -jnp.einsum('...ij,...j->...i', Rt, p))


def _adjoint(T):
    R = T[..., :3, :3]; p = T[..., :3, 3]
    Z = jnp.zeros_like(R)
    return jnp.concatenate([jnp.concatenate([R, Z], -1),
                            jnp.concatenate([_skew(p) @ R, R], -1)], -2)


def _ad(V):
    wh = _skew(V[..., :3]); vh = _skew(V[..., 3:])
    Z = jnp.zeros_like(wh)
    return jnp.concatenate([jnp.concatenate([wh, Z], -1),
                            jnp.concatenate([vh, wh], -1)], -2)


def _screw_exp(S, theta):
    w, v = S[:3], S[3:]
    wh = _skew(w); wh2 = wh @ wh
    th = theta[:, None, None]
    s, c = jnp.sin(th), jnp.cos(th)
    I3 = jnp.eye(3, dtype=theta.dtype)
    R = I3 + s * wh + (1. - c) * wh2
    G = I3 * th + (1. - c) * wh + (th - s) * wh2
    return _make_T(R, jnp.einsum('bij,j->bi', G, v))


def _build_M(M_raw):
    a1 = M_raw[:, :3, 0]; a2 = M_raw[:, :3, 1]
    b1 = a1 / jnp.linalg.norm(a1, axis=-1, keepdims=True)
    a2p = a2 - jnp.sum(a2 * b1, -1, keepdims=True) * b1
    b2 = a2p / jnp.linalg.norm(a2p, axis=-1, keepdims=True)
    b3 = jnp.cross(b1, b2)
    return _make_T(jnp.stack([b1, b2, b3], -1), M_raw[:, :3, 3])


def _build_A(A_raw):
    w = A_raw[:, :3]
    w = w / jnp.linalg.norm(w, axis=-1, keepdims=True)
    return jnp.concatenate([w, A_raw[:, 3:]], -1)


def _build_G(G_raw):
    n = G_raw.shape[0]; dt = G_raw.dtype
    idx3 = jnp.array([0, 1, 2])
    L = jnp.zeros((n, 3, 3), dt)
    L = L.at[:, idx3, idx3].set(jnp.abs(G_raw[:, :3]))
    L = L.at[:, jnp.array([1, 2, 2]), jnp.array([0, 0, 1])].set(G_raw[:, 3:6])
    p = _skew(G_raw[:, 7:])
    G = jnp.zeros((n, 6, 6), dt)
    G = G.at[:, :3, :3].set(L @ jnp.swapaxes(L, -1, -2))
    G = G.at[:, jnp.array([3, 4, 5]), jnp.array([3, 4, 5])].set(jnp.abs(G_raw[:, 6])[:, None])
    G = G.at[:, :3, 3:].set(p)
    G = G.at[:, 3:, :3].set(jnp.swapaxes(p, -1, -2))
    return G


def _forward_dynamics(q, dq, tau, gravity, ftip, Mlist, Glist, A):
    n, B, dt = A.shape[0], q.shape[0], q.dtype
    AdTs = [_adjoint(_screw_exp(A[i], -q[:, i]) @ _inv_SE3(Mlist[i])) for i in range(n)]
    AdT_end = _adjoint(_inv_SE3(Mlist[n]))

    def inv_dyn(dq_, ddq_, g, ft):
        V = jnp.zeros((B, 6), dt)
        Vd = jnp.broadcast_to(jnp.concatenate([jnp.zeros(3, dt), -g]), (B, 6))
        Vs, Vds = [], []
        for i in range(n):
            V = jnp.einsum('bij,bj->bi', AdTs[i], V) + A[i] * dq_[:, i, None]
            Vd = (jnp.einsum('bij,bj->bi', AdTs[i], Vd)
                  + jnp.einsum('bij,j->bi', _ad(V), A[i]) * dq_[:, i, None]
                  + A[i] * ddq_[:, i, None])
            Vs.append(V); Vds.append(Vd)
        F = jnp.broadcast_to(ft, (B, 6))
        taus = [None] * n
        for i in range(n - 1, -1, -1):
            if i == n - 1:
                F = jnp.einsum('ji,bj->bi', AdT_end, F)
            else:
                F = jnp.einsum('bji,bj->bi', AdTs[i + 1], F)
            GV = jnp.einsum('ij,bj->bi', Glist[i], Vs[i])
            F = F + jnp.einsum('ij,bj->bi', Glist[i], Vds[i]) - jnp.einsum('bji,bj->bi', _ad(Vs[i]), GV)
            taus[i] = jnp.sum(F * A[i], -1)
        return jnp.stack(taus, -1)

    z_n = jnp.zeros((B, n), dt); z3 = jnp.zeros(3, dt); z6 = jnp.zeros(6, dt)
    cols = jax.vmap(lambda e: inv_dyn(z_n, jnp.broadcast_to(e, (B, n)), z3, z6))(jnp.eye(n, dtype=dt))
    Mmat = jnp.transpose(cols, (1, 2, 0))
    h = inv_dyn(dq, z_n, gravity, ftip)
    return _solve_spd(Mmat, tau - h)


def _solve_spd(Mmat, rhs):
    # Gaussian elimination without pivoting (SPD mass matrix), vectorized
    # over the batch; lowers to plain elementwise ops.
    n = rhs.shape[-1]
    M = [[Mmat[:, r, c] for c in range(n)] for r in range(n)]
    b = [rhs[:, r] for r in range(n)]
    for k in range(n):
        inv = 1.0 / M[k][k]
        for r in range(k + 1, n):
            f = M[r][k] * inv
            for c in range(k + 1, n):
                M[r][c] = M[r][c] - f * M[k][c]
            b[r] = b[r] - f * b[k]
    x = [None] * n
    for r in range(n - 1, -1, -1):
        acc = b[r]
        for c in range(r + 1, n):
            acc = acc - M[r][c] * x[c]
        x[r] = acc / M[r][r]
    return jnp.stack(x, -1)


def _fk_in_space(q, Mlist, A):
    n, B = A.shape[0], q.shape[0]
    T = jnp.broadcast_to(jnp.eye(4, dtype=q.dtype), (B, 4, 4))
    frames = []
    for i in range(n):
        T = (T @ Mlist[i]) @ _screw_exp(A[i], q[:, i])
        frames.append(T)
    frames.append(T @ Mlist[n])
    return jnp.stack(frames, 1)


def _rk4_step(f, y0, h):
    k1 = f(y0)
    k2 = f(y0 + 0.5 * h * k1)
    k3 = f(y0 + 0.5 * h * k2)
    k4 = f(y0 + h * k3)
    return y0 + (h / 6.) * (k1 + 2. * k2 + 2. * k3 + k4)


def _forward_shard(state, action, M_raw, A_raw, G_raw, gravity, ftip):
    n = DOF
    Mlist, Glist, A = _build_M(M_raw), _build_G(G_raw), _build_A(A_raw)
    torque = action * ACTION_RANGE

    def derivs(s):
        qpos, qvel = s[..., :n], s[..., n:2 * n]
        qf = s[..., 2 * n:3 * n] - DAMPING * qvel
        qacc = _forward_dynamics(qpos, qvel, qf, gravity, ftip, Mlist, Glist, A)
        return jnp.concatenate([qvel, qacc, jnp.zeros_like(qf)], -1)

    s1 = _rk4_step(derivs, jnp.concatenate([state, torque], -1), TIMESTEP)
    q = (s1[..., :n] + jnp.pi) % (2. * jnp.pi) - jnp.pi
    dq = jnp.clip(s1[..., n:2 * n], -MAX_VEL, MAX_VEL)
    ee = _fk_in_space(q, Mlist, A)
    return jnp.concatenate([q, dq], -1), ee[:, -1, :2, 3]


@functools.partial(jax.pmap, in_axes=(0, 0, None, None, None, None, None))
def _forward_pmap(state, action, M_raw, A_raw, G_raw, gravity, ftip):
    return _forward_shard(state, action, M_raw, A_raw, G_raw, gravity, ftip)


def kernel(state, action, M_raw, A_raw, G_raw, gravity, ftip):
    state = np.asarray(state, dtype=np.float32)
    action = np.asarray(action, dtype=np.float32)
    B = state.shape[0]
    per = B // N_CORES
    st = state.reshape(N_CORES, per, state.shape[1])
    ac = action.reshape(N_CORES, per, action.shape[1])
    args = (st, ac,
            jnp.asarray(M_raw, jnp.float32), jnp.asarray(A_raw, jnp.float32),
            jnp.asarray(G_raw, jnp.float32), jnp.asarray(gravity, jnp.float32),
            jnp.asarray(ftip, jnp.float32))
    last = None
    for attempt in range(3):
        try:
            out_s, out_ee = _forward_pmap(*args)
            out_s = np.asarray(out_s).reshape(B, -1)
            out_ee = np.asarray(out_ee).reshape(B, -1)
            return out_s, out_ee
        except Exception as e:  # transient NRT exec-unit failures on cold devices
            last = e
            import time as _t
            _t.sleep(5.0)
    raise last
